# revision 46
# baseline (speedup 1.0000x reference)
"""GRU (Flax GRUCell scanned over time) on 8 Trainium2 NeuronCores.

Problem: x:[T,B,D]=[512,64,512], h0:[B,H], Wi:[D,3H], Wh:[H,3H], bi:[3H], bhn:[H]
  gi = x_t @ Wi + bi ; gh = h @ Wh ; gates (r,z,n); h' = (1-z)*n + z*h
  returns ys:[T,B,H] (the h trajectory).

Strategy (per core, data-parallel over batch, B_local=8):
  Everything on-chip lives in "T-layout": hidden dim on SBUF partitions,
  batch on the free dim.

  Phase 1 (one-time, ~105us): gi = x @ Wi for ALL T steps as dense
  N=512 matmuls (weight loads amortized 64x vs per-step), stored fp16
  in SBUF (~96KB/partition).  Running the whole prologue upfront also
  completes the DVFS clock ramp, so the latency-bound scan loop never
  sees reduced clocks.

  Phase 2 (the scan), per step (~3.17us steady):
    - gi_rz is injected into a fresh psum bank by ONE identity matmul
      (gp += I.T @ gi_rz, issued PF steps ahead; no hb dependency so it
      rides the PE idle window; being a PE write it is race-free against
      the accumulating gh matmuls, unlike a DVE prefill copy).
    - 48 gh matmuls accumulate on top (r/z first so their groups close
      ~450ns before the n rows; the PE-sem visibility wait ~450ns).
    - critical chain: sigmoid_r(psum) -> rpn = ps*r -> pre_n = rpn+gi_n
      -> sig2 = sigmoid(2*pre_n) [= (tanh+1)/2] -> ONE tensor_tensor_scan
      computing h' = a*sig2 + b over interleaved pairs -> hb (bf16).
      a = 2-2z and b = z*h-1+z are built off-chain (GpSimd/DVE) after
      sigmoid_z; the fp32 h' for the output ring is recomputed on GpSimd.
  h state stays fp32; output is written transposed and reassembled on host.
"""

import warnings

warnings.filterwarnings("ignore")

import numpy as np
import ml_dtypes

import concourse.bacc as bacc
import concourse.tile as tile
from concourse import mybir, bass_utils

B, D, H = 64, 512, 512
NCORES = 8
BL = B // NCORES  # batch per core
KD = D // 128  # input-dim k-chunks
KH = H // 128  # hidden-dim k-chunks
M3 = 3 * H // 128  # 3H m-tiles
RT = 8  # output-ring steps per DMA
PF = 2  # gi_rz psum prefill lead (steps)
CW = 512  # precompute chunk width (psum bank = 512 f32)
BF16 = mybir.dt.bfloat16
F16 = mybir.dt.float16
F32 = mybir.dt.float32
NPBF16 = ml_dtypes.bfloat16

_cache: dict = {}


def _build(T: int, use_bi: bool, use_bhn: bool):
    TB = T * BL
    assert T % RT == 0 and TB % CW == 0
    CH = TB // CW  # precompute chunks
    nc = bacc.Bacc("TRN2", target_bir_lowering=False, debug=False, num_devices=NCORES)

    xt_d = nc.dram_tensor("xt", [128, KD * TB], BF16, kind="ExternalInput").ap()
    wi_d = nc.dram_tensor("wi", [128, M3 * KD * 128], BF16, kind="ExternalInput").ap()
    wh_d = nc.dram_tensor("wh", [128, M3 * KH * 128], BF16, kind="ExternalInput").ap()
    h0_d = nc.dram_tensor("h0t", [128, KH * BL], F32, kind="ExternalInput").ap()
    bi_d = (
        nc.dram_tensor("bi_r", [1, M3 * 128], BF16, kind="ExternalInput").ap()
        if use_bi
        else None
    )
    bhn_d = (
        nc.dram_tensor("bhn_t", [128, KH], F32, kind="ExternalInput").ap()
        if use_bhn
        else None
    )
    eye_d = nc.dram_tensor("eye", [128, 128], F16, kind="ExternalInput").ap()
    ys_d = nc.dram_tensor("yst", [128, KH * TB], F32, kind="ExternalOutput").ap()
    ys_v = ys_d.rearrange("p (k t j) -> p k t j", k=KH, j=BL)

    with tile.TileContext(nc) as tc:
        with (
            tc.tile_pool(name="const", bufs=1) as const,
            tc.tile_pool(name="gib", bufs=1) as gib,
            tc.tile_pool(name="xin", bufs=1) as xin,
            tc.tile_pool(name="pre", bufs=4, space="PSUM") as pre,
            tc.tile_pool(name="whps", bufs=1, space="PSUM") as whps,
            tc.tile_pool(name="gps", bufs=PF + 1, space="PSUM") as gps,
            tc.tile_pool(name="orp", bufs=3) as orp,
            tc.tile_pool(name="hbp", bufs=3) as hbp,
            tc.tile_pool(name="ew", bufs=2) as ew,
        ):
            # ---- load constants ----
            wi_sb = const.tile([128, M3 * KD * 128], BF16)
            nc.sync.dma_start(wi_sb[:], wi_d[:])
            wh_sb = const.tile([128, M3 * KH * 128], BF16)
            nc.sync.dma_start(wh_sb[:], wh_d[:])
            h0_sb = const.tile([128, KH, BL], F32)
            nc.sync.dma_start(h0_sb[:], h0_d.rearrange("p (k j) -> p k j", j=BL))
            eye_sb = const.tile([128, 128], F16)
            nc.sync.dma_start(eye_sb[:], eye_d[:])
            if use_bi:
                bi_sb = const.tile([1, M3 * 128], BF16)
                nc.sync.dma_start(bi_sb[:], bi_d[:])
                ones_sb = const.tile([1, CW], BF16)
                nc.vector.memset(ones_sb[:], 1.0)
            if use_bhn:
                bhn_sb = const.tile([128, KH], F32)
                nc.sync.dma_start(bhn_sb[:], bhn_d[:])
            xt_sb = xin.tile([128, KD * TB], BF16)

            def load_x_chunk(c):
                for k in range(KD):
                    s = k * TB + c * CW
                    nc.sync.dma_start(xt_sb[:, s : s + CW], xt_d[:, s : s + CW])

            # ---- phase 1: dense gi precompute into SBUF (fp16: bf16 would
            # cost ~1e-2 rel err through the recurrence; fp16's 10-bit
            # mantissa keeps the gi rounding ~8x smaller at the same size).
            # gi_rz keeps m-major layout (prefill reads per-t strided);
            # gi_n is stored t-major so pre_n's per-step read is contiguous.
            gi_sb = gib.tile([128, 8, TB], F16)
            gn_sb = gib.tile([128, T, KH, BL], F32)
            TC = CW // BL  # timesteps per precompute chunk

            _pp_cur = [None]

            def gi_mm(c, m, k, in_loop):
                """One matmul of chunk c's m-tile accumulation (k-th of KD)."""
                if k == 0:
                    _pp_cur[0] = pre.tile([128, CW], F32, tag="pre", name="pp")
                pp = _pp_cur[0]
                nc.tensor.matmul(
                    pp[:],
                    wi_sb[:, (m * KD + k) * 128 : (m * KD + k + 1) * 128],
                    xt_sb[:, k * TB + c * CW : k * TB + (c + 1) * CW],
                    start=(k == 0),
                    stop=(k == KD - 1) and not use_bi,
                    skip_group_check=True,
                )
                if k < KD - 1:
                    return
                if use_bi:
                    nc.tensor.matmul(
                        pp[:],
                        bi_sb[:, m * 128 : (m + 1) * 128],
                        ones_sb[:],
                        start=False,
                        stop=True,
                        skip_group_check=True,
                    )
                if m < 8:
                    dst = gi_sb[:, m, c * CW : (c + 1) * CW]
                    src = pp[:]
                else:
                    dst = gn_sb[:, c * TC : (c + 1) * TC, m - 8, :]
                    src = pp.rearrange("p (t j) -> p t j", j=BL)
                # in-loop copies ride the ACT queue after sig2; upfront ones
                # alternate engines so they hide behind the matmuls
                if in_loop or m % 2:
                    nc.scalar.copy(dst, src)
                else:
                    nc.vector.tensor_copy(dst, src)

            # ALL chunks upfront (UPC=CH): the ~100us dense prologue also
            # completes the DVFS clock ramp, so the latency-sensitive scan
            # loop never runs at reduced clocks.  (Streaming chunks into the
            # loop was tried: the loop then starts inside the ramp and the
            # first ~35 steps run ~2.5x slow; bursty streaming additionally
            # trips DVFS throttling for the whole run.)
            UPC = CH
            for c in range(UPC + 1):
                if c < CH:
                    load_x_chunk(c)
            for c in range(UPC):
                for m in range(M3):
                    for k in range(KD):
                        gi_mm(c, m, k, False)

            # in-loop streaming schedule: step -> list of emissions
            stream_mm: dict = {}
            stream_dma: dict = {}
            for c in range(UPC, CH):
                base = TC * (c - 1) + 2
                for m in range(M3):
                    for k in range(KD):
                        stream_mm.setdefault(base + 4 * m + k, []).append((c, m, k))
                if c + 1 < CH:
                    stream_dma.setdefault(TC * (c - 1) + 1, []).append(c + 1)

            gi_v = gi_sb.rearrange("p m (t j) -> p m t j", j=BL)

            # ---- phase 2: the scan ----
            def gp_inject(tt):
                """gi_rz for step tt -> fresh psum bank via identity matmuls:
                gp[:, m, :] += I.T @ gi_rz[m].  All-PE writes, so the gh
                accumulation order is guaranteed by the PE FIFO (a DVE
                prefill copy raced the accumulating matmuls).  These have no
                hb dependency, so they run in the PE idle window."""
                g = gps.tile([128, 8, BL], F32, tag="gp")
                nc.tensor.matmul(
                    g[:, :, :],
                    eye_sb[:],
                    gi_v[:, :, tt, :],
                    start=True,
                    stop=False,
                    skip_group_check=True,
                )
                return g

            pend = [gp_inject(tt) for tt in range(min(PF, T))]

            # Interleaved pair tiles for the blend scan: per (row, batch) pair
            # j: state0 = 0*junk + a, state1 = sig2*a + b = h'.
            # d0 = [0, sig2], d1 = [a, b]; hb = odd lanes of the scan output.
            d0_tiles = []
            for i in range(2):
                d0 = const.tile([128, KH, BL, 2], F32, name=f"d0_{i}")
                nc.vector.memset(d0[:, :, :, 0], 0.0)
                d0_tiles.append(d0)

            hb4 = hbp.tile([128, KH, BL, 2], BF16, tag="hb")
            nc.vector.memset(hb4[:], 0.0)
            nc.vector.tensor_copy(hb4[:, :, :, 1], h0_sb[:])
            hb = hb4[:, :, :, 1]
            h_prev = h0_sb[:, :, :]

            # gh matmul issue order: r/z closes at #32 and the n psum at #48,
            # balancing sigma's PE-vis wait against rpn's ps-vis wait.
            MMSEQ = (
                [(m, k) for m in range(8) for k in (0, 1, 2, 3)]
                + [(m, k) for m in range(8, 12) for k in (0, 1, 2, 3)]
            )

            o_cur = None
            for t in range(T):
                u = t % RT
                if u == 0:
                    o_cur = orp.tile([128, KH, RT, BL], F32, tag="oring")

                gp = pend.pop(0)
                # inject for t+PF is emitted BEFORE the matmul burst: its
                # identity matmuls have no hb dep and fill the idle window
                if t + PF < T:
                    pend.append(gp_inject(t + PF))

                # on-chain: ghT matmuls (need h from last step).
                # r/z rows accumulate onto the prefilled gi_rz psum; n rows
                # into their own psum so r can gate gh_n alone.
                ps = whps.tile([128, KH, BL], F32, tag="whp")
                for m, k in MMSEQ:
                    out_ap = gp[:, m, :] if m < 8 else ps[:, m - 8, :]
                    nc.tensor.matmul(
                        out_ap,
                        wh_sb[:, (m * KH + k) * 128 : (m * KH + k + 1) * 128],
                        hb[:, k, :],
                        start=(m == 8 and k == 0),
                        stop=(k == KH - 1),
                        skip_group_check=True,
                    )

                # Gate math.  Critical chain:
                #   sigmoid_r(psum) -> rpn -> pre_n -> sig2 -> scan -> hb
                # where n = tanh(pre_n) = 2*sig2 - 1 with sig2 = sigmoid(2*pre_n),
                # so h' = (1-z)*n + z*h = a*sig2 + b with a = 2-2z and
                # b = z*h - 1 + z = z*(h+1) - 1 = z*hp1 - 1 (hp1 = h+1 was
                # computed last step, so b is 2 off-chain GpSimd ops after
                # sigmoid_z; a is one off-chain DVE op).  The blend a*sig2+b
                # is ONE DVE tensor_tensor_scan over interleaved pairs.
                rzt = ew.tile([128, 8, BL], F32, tag="rzt")
                nc.scalar.activation(
                    rzt[:, 0:KH, :], gp[:, 0:KH, :],
                    mybir.ActivationFunctionType.Sigmoid,
                )
                nc.scalar.activation(
                    rzt[:, KH : 2 * KH, :], gp[:, KH : 2 * KH, :],
                    mybir.ActivationFunctionType.Sigmoid,
                )
                d1 = ew.tile([128, KH, BL, 2], F32, tag="d1")
                zt = rzt[:, KH : 2 * KH, :]
                v = ew.tile([128, KH, BL], F32, tag="v")
                nc.gpsimd.tensor_mul(v[:], zt, h_prev)
                rpn = ew.tile([128, KH, BL], F32, tag="rpn")
                pre_n = ew.tile([128, KH, BL], F32, tag="pren")
                if use_bhn:
                    for k in range(KH):
                        nc.vector.scalar_tensor_tensor(
                            rpn[:, k, :],
                            ps[:, k, :],
                            bhn_sb[:, k : k + 1],
                            rzt[:, k, :],
                            mybir.AluOpType.add,
                            mybir.AluOpType.mult,
                        )
                    nc.vector.tensor_add(pre_n[:], rpn[:], gn_sb[:, t, :, :])
                else:
                    nc.vector.tensor_mul(rpn[:], ps[:], rzt[:, 0:KH, :])
                    nc.vector.tensor_add(pre_n[:], rpn[:], gn_sb[:, t, :, :])
                nc.vector.tensor_scalar(
                    d1[:, :, :, 0], zt, -2.0, 2.0,
                    mybir.AluOpType.mult, mybir.AluOpType.add,
                )
                # b = v - 1 + z on DVE (scalar_tensor_tensor is DVE-only)
                nc.vector.scalar_tensor_tensor(
                    d1[:, :, :, 1], v[:], -1.0, zt,
                    mybir.AluOpType.add, mybir.AluOpType.add,
                )
                d0 = d0_tiles[t % 2]
                nc.scalar.activation(
                    d0[:, :, :, 1], pre_n[:],
                    mybir.ActivationFunctionType.Sigmoid, scale=2.0,
                )
                hb4 = hbp.tile([128, KH, BL, 2], BF16, tag="hb")
                nc.vector.tensor_tensor_scan(
                    hb4.rearrange("p k j i -> p (k j i)"),
                    d0.rearrange("p k j i -> p (k j i)"),
                    d1.rearrange("p k j i -> p (k j i)"),
                    0.0,
                    mybir.AluOpType.mult,
                    mybir.AluOpType.add,
                )
                hb = hb4[:, :, :, 1]
                h_new = o_cur[:, :, u, :]
                # fp32 h for output/next-step b, off the critical chain:
                # h' = a*sig2 + b recomputed at f32 on GpSimd so these ops
                # never sit ahead of the scan in the DVE queue
                wq = ew.tile([128, KH, BL], F32, tag="wq")
                nc.gpsimd.tensor_mul(wq[:], d0[:, :, :, 1], d1[:, :, :, 0])
                nc.gpsimd.tensor_add(h_new, wq[:], d1[:, :, :, 1])
                h_prev = h_new

                # streamed gi precompute (chunks 1..CH-1) rides the PE/ACT
                # idle window; x-chunk DMAs issue a chunk ahead
                for c in stream_dma.get(t, ()):
                    load_x_chunk(c)
                for c, m, k in stream_mm.get(t, ()):
                    gi_mm(c, m, k, True)

                if u == RT - 1:
                    nc.sync.dma_start(
                        ys_v[:, :, t - RT + 1 : t + 1, :], o_cur[:]
                    )

    nc.compile()
    return nc


def _get(T, use_bi, use_bhn):
    key = (T, use_bi, use_bhn)
    if key not in _cache:
        _cache[key] = _build(T, use_bi, use_bhn)
    return _cache[key]


def _pack_w(W, kc):
    # W [kc*128, M3*128] -> [128, M3*kc*128], col ((m*kc)+k)*128+c = W[k*128+p, m*128+c]
    return np.ascontiguousarray(
        W.astype(NPBF16).reshape(kc, 128, M3, 128).transpose(1, 2, 0, 3).reshape(128, -1)
    )


def kernel(x, h0, Wi, Wh, bi, bhn, _trace=False, _trace_kwargs=None):
    T = x.shape[0]
    use_bi = bool(np.any(bi))
    use_bhn = bool(np.any(bhn))
    nc = _get(T, use_bi, use_bhn)
    TB = T * BL

    wi_p = _pack_w(np.asarray(Wi), KD)
    wh_p = _pack_w(np.asarray(Wh), KH)
    x = np.asarray(x)
    h0 = np.asarray(h0)

    in_maps = []
    for c in range(NCORES):
        xc = x[:, c * BL : (c + 1) * BL, :]  # [T, BL, D]
        xt = np.ascontiguousarray(
            xc.astype(NPBF16).reshape(T, BL, KD, 128).transpose(3, 2, 0, 1).reshape(128, KD * TB)
        )
        h0c = np.ascontiguousarray(
            h0[c * BL : (c + 1) * BL, :].astype(np.float32).reshape(BL, KH, 128).transpose(2, 1, 0).reshape(128, KH * BL)
        )
        im = {
            "xt": xt,
            "wi": wi_p,
            "wh": wh_p,
            "h0t": h0c,
            "eye": np.eye(128, dtype=np.float16),
        }
        if use_bi:
            im["bi_r"] = np.ascontiguousarray(bi.astype(NPBF16).reshape(1, M3 * 128))
        if use_bhn:
            im["bhn_t"] = np.ascontiguousarray(bhn.astype(np.float32).reshape(KH, 128).T)
        in_maps.append(im)

    kw = {}
    if _trace:
        kw = dict(trace=True, **(_trace_kwargs or {}))
    kernel._last_in_maps = in_maps
    res = bass_utils.run_bass_kernel_spmd(nc, in_maps, core_ids=list(range(NCORES)), **kw)

    ys = np.empty((T, B, H), dtype=np.float32)
    for c in range(NCORES):
        out = res.results[c]["yst"]  # [128, KH*TB]
        ys[:, c * BL : (c + 1) * BL, :] = (
            out.reshape(128, KH, T, BL).transpose(2, 3, 1, 0).reshape(T, BL, H)
        )
    kernel._last_result = res
    return ys


# revision 47
# speedup vs baseline: 1.0014x; 1.0014x over previous
"""GRU (Flax GRUCell scanned over time) on 8 Trainium2 NeuronCores.

Problem: x:[T,B,D]=[512,64,512], h0:[B,H], Wi:[D,3H], Wh:[H,3H], bi:[3H], bhn:[H]
  gi = x_t @ Wi + bi ; gh = h @ Wh ; gates (r,z,n); h' = (1-z)*n + z*h
  returns ys:[T,B,H] (the h trajectory).

Strategy (per core, data-parallel over batch, B_local=8):
  Everything on-chip lives in "T-layout": hidden dim on SBUF partitions,
  batch on the free dim.

  Phase 1 (one-time, ~105us): gi = x @ Wi for ALL T steps as dense
  N=512 matmuls (weight loads amortized 64x vs per-step), stored fp16
  in SBUF (~96KB/partition).  Running the whole prologue upfront also
  completes the DVFS clock ramp, so the latency-bound scan loop never
  sees reduced clocks.

  Phase 2 (the scan), per step (~3.17us steady):
    - gi_rz is injected into a fresh psum bank by ONE identity matmul
      (gp += I.T @ gi_rz, issued PF steps ahead; no hb dependency so it
      rides the PE idle window; being a PE write it is race-free against
      the accumulating gh matmuls, unlike a DVE prefill copy).
    - 48 gh matmuls accumulate on top (r/z first so their groups close
      ~450ns before the n rows; the PE-sem visibility wait ~450ns).
    - critical chain: sigmoid_r(psum) -> rpn = ps*r -> pre_n = rpn+gi_n
      -> sig2 = sigmoid(2*pre_n) [= (tanh+1)/2] -> ONE tensor_tensor_scan
      computing h' = a*sig2 + b over interleaved pairs -> hb (bf16).
      a = 2-2z and b = z*h-1+z are built off-chain (GpSimd/DVE) after
      sigmoid_z; the fp32 h' for the output ring is recomputed on GpSimd.
  h state stays fp32; output is written transposed and reassembled on host.
"""

import warnings

warnings.filterwarnings("ignore")

import numpy as np
import ml_dtypes

import concourse.bacc as bacc
import concourse.tile as tile
from concourse import mybir, bass_utils

B, D, H = 64, 512, 512
NCORES = 8
BL = B // NCORES  # batch per core
KD = D // 128  # input-dim k-chunks
KH = H // 128  # hidden-dim k-chunks
M3 = 3 * H // 128  # 3H m-tiles
RT = 8  # output-ring steps per DMA
PF = 2  # gi_rz psum prefill lead (steps)
CW = 512  # precompute chunk width (psum bank = 512 f32)
BF16 = mybir.dt.bfloat16
F16 = mybir.dt.float16
F32 = mybir.dt.float32
NPBF16 = ml_dtypes.bfloat16

_cache: dict = {}


def _build(T: int, use_bi: bool, use_bhn: bool):
    TB = T * BL
    assert T % RT == 0 and TB % CW == 0
    CH = TB // CW  # precompute chunks
    nc = bacc.Bacc("TRN2", target_bir_lowering=False, debug=False, num_devices=NCORES)

    xt_d = nc.dram_tensor("xt", [128, KD * TB], BF16, kind="ExternalInput").ap()
    wi_d = nc.dram_tensor("wi", [128, M3 * KD * 128], BF16, kind="ExternalInput").ap()
    wh_d = nc.dram_tensor("wh", [128, M3 * KH * 128], BF16, kind="ExternalInput").ap()
    h0_d = nc.dram_tensor("h0t", [128, KH * BL], F32, kind="ExternalInput").ap()
    bi_d = (
        nc.dram_tensor("bi_r", [1, M3 * 128], BF16, kind="ExternalInput").ap()
        if use_bi
        else None
    )
    bhn_d = (
        nc.dram_tensor("bhn_t", [128, KH], F32, kind="ExternalInput").ap()
        if use_bhn
        else None
    )
    eye_d = nc.dram_tensor("eye", [128, 128], F16, kind="ExternalInput").ap()
    ys_d = nc.dram_tensor("yst", [128, KH * TB], F32, kind="ExternalOutput").ap()
    ys_v = ys_d.rearrange("p (k t j) -> p k t j", k=KH, j=BL)

    with tile.TileContext(nc) as tc:
        with (
            tc.tile_pool(name="const", bufs=1) as const,
            tc.tile_pool(name="gib", bufs=1) as gib,
            tc.tile_pool(name="xin", bufs=1) as xin,
            tc.tile_pool(name="pre", bufs=3, space="PSUM") as pre,
            tc.tile_pool(name="whps", bufs=2, space="PSUM") as whps,
            tc.tile_pool(name="gps", bufs=PF + 1, space="PSUM") as gps,
            tc.tile_pool(name="orp", bufs=3) as orp,
            tc.tile_pool(name="hbp", bufs=3) as hbp,
            tc.tile_pool(name="ew", bufs=2) as ew,
        ):
            # ---- load constants ----
            wi_sb = const.tile([128, M3 * KD * 128], BF16)
            nc.sync.dma_start(wi_sb[:], wi_d[:])
            wh_sb = const.tile([128, M3 * KH * 128], BF16)
            nc.sync.dma_start(wh_sb[:], wh_d[:])
            h0_sb = const.tile([128, KH, BL], F32)
            nc.sync.dma_start(h0_sb[:], h0_d.rearrange("p (k j) -> p k j", j=BL))
            eye_sb = const.tile([128, 128], F16)
            nc.sync.dma_start(eye_sb[:], eye_d[:])
            if use_bi:
                bi_sb = const.tile([1, M3 * 128], BF16)
                nc.sync.dma_start(bi_sb[:], bi_d[:])
                ones_sb = const.tile([1, CW], BF16)
                nc.vector.memset(ones_sb[:], 1.0)
            if use_bhn:
                bhn_sb = const.tile([128, KH], F32)
                nc.sync.dma_start(bhn_sb[:], bhn_d[:])
            xt_sb = xin.tile([128, KD * TB], BF16)

            def load_x_chunk(c):
                for k in range(KD):
                    s = k * TB + c * CW
                    nc.sync.dma_start(xt_sb[:, s : s + CW], xt_d[:, s : s + CW])

            # ---- phase 1: dense gi precompute into SBUF (fp16: bf16 would
            # cost ~1e-2 rel err through the recurrence; fp16's 10-bit
            # mantissa keeps the gi rounding ~8x smaller at the same size).
            # gi_rz keeps m-major layout (prefill reads per-t strided);
            # gi_n is stored t-major so pre_n's per-step read is contiguous.
            gi_sb = gib.tile([128, 8, TB], F16)
            gn_sb = gib.tile([128, T, KH, BL], F32)
            TC = CW // BL  # timesteps per precompute chunk

            _pp_cur = [None]

            def gi_mm(c, m, k, in_loop):
                """One matmul of chunk c's m-tile accumulation (k-th of KD)."""
                if k == 0:
                    _pp_cur[0] = pre.tile([128, CW], F32, tag="pre", name="pp")
                pp = _pp_cur[0]
                nc.tensor.matmul(
                    pp[:],
                    wi_sb[:, (m * KD + k) * 128 : (m * KD + k + 1) * 128],
                    xt_sb[:, k * TB + c * CW : k * TB + (c + 1) * CW],
                    start=(k == 0),
                    stop=(k == KD - 1) and not use_bi,
                    skip_group_check=True,
                )
                if k < KD - 1:
                    return
                if use_bi:
                    nc.tensor.matmul(
                        pp[:],
                        bi_sb[:, m * 128 : (m + 1) * 128],
                        ones_sb[:],
                        start=False,
                        stop=True,
                        skip_group_check=True,
                    )
                if m < 8:
                    dst = gi_sb[:, m, c * CW : (c + 1) * CW]
                    src = pp[:]
                else:
                    dst = gn_sb[:, c * TC : (c + 1) * TC, m - 8, :]
                    src = pp.rearrange("p (t j) -> p t j", j=BL)
                # in-loop copies ride the ACT queue after sig2; upfront ones
                # alternate engines so they hide behind the matmuls
                if in_loop or m % 2:
                    nc.scalar.copy(dst, src)
                else:
                    nc.vector.tensor_copy(dst, src)

            # ALL chunks upfront (UPC=CH): the ~100us dense prologue also
            # completes the DVFS clock ramp, so the latency-sensitive scan
            # loop never runs at reduced clocks.  (Streaming chunks into the
            # loop was tried: the loop then starts inside the ramp and the
            # first ~35 steps run ~2.5x slow; bursty streaming additionally
            # trips DVFS throttling for the whole run.)
            UPC = CH
            for c in range(UPC + 1):
                if c < CH:
                    load_x_chunk(c)
            for c in range(UPC):
                for m in range(M3):
                    for k in range(KD):
                        gi_mm(c, m, k, False)

            # in-loop streaming schedule: step -> list of emissions
            stream_mm: dict = {}
            stream_dma: dict = {}
            for c in range(UPC, CH):
                base = TC * (c - 1) + 2
                for m in range(M3):
                    for k in range(KD):
                        stream_mm.setdefault(base + 4 * m + k, []).append((c, m, k))
                if c + 1 < CH:
                    stream_dma.setdefault(TC * (c - 1) + 1, []).append(c + 1)

            gi_v = gi_sb.rearrange("p m (t j) -> p m t j", j=BL)

            # ---- phase 2: the scan ----
            def gp_inject(tt):
                """gi_rz for step tt -> fresh psum bank via identity matmuls:
                gp[:, m, :] += I.T @ gi_rz[m].  All-PE writes, so the gh
                accumulation order is guaranteed by the PE FIFO (a DVE
                prefill copy raced the accumulating matmuls).  These have no
                hb dependency, so they run in the PE idle window."""
                g = gps.tile([128, 8, BL], F32, tag="gp")
                nc.tensor.matmul(
                    g[:, :, :],
                    eye_sb[:],
                    gi_v[:, :, tt, :],
                    start=True,
                    stop=False,
                    skip_group_check=True,
                )
                return g

            pend = [gp_inject(tt) for tt in range(min(PF, T))]

            # Interleaved pair tiles for the blend scan: per (row, batch) pair
            # j: state0 = 0*junk + a, state1 = sig2*a + b = h'.
            # d0 = [0, sig2], d1 = [a, b]; hb = odd lanes of the scan output.
            d0_tiles = []
            for i in range(2):
                d0 = const.tile([128, KH, BL, 2], F32, name=f"d0_{i}")
                nc.vector.memset(d0[:, :, :, 0], 0.0)
                d0_tiles.append(d0)

            hb4 = hbp.tile([128, KH, BL, 2], BF16, tag="hb")
            nc.vector.memset(hb4[:], 0.0)
            nc.vector.tensor_copy(hb4[:, :, :, 1], h0_sb[:])
            hb = hb4[:, :, :, 1]
            h_prev = h0_sb[:, :, :]

            # gh matmul issue order: r/z closes at #32 and the n psum at #48,
            # balancing sigma's PE-vis wait against rpn's ps-vis wait.
            MMSEQ = (
                [(m, k) for m in range(8) for k in (0, 1, 2, 3)]
                + [(m, k) for m in range(8, 12) for k in (0, 1, 2, 3)]
            )

            o_cur = None
            for t in range(T):
                u = t % RT
                if u == 0:
                    o_cur = orp.tile([128, KH, RT, BL], F32, tag="oring")

                gp = pend.pop(0)
                # inject for t+PF is emitted BEFORE the matmul burst: its
                # identity matmuls have no hb dep and fill the idle window
                if t + PF < T:
                    pend.append(gp_inject(t + PF))

                # on-chain: ghT matmuls (need h from last step).
                # r/z rows accumulate onto the prefilled gi_rz psum; n rows
                # into their own psum so r can gate gh_n alone.
                ps = whps.tile([128, KH, BL], F32, tag="whp")
                for m, k in MMSEQ:
                    out_ap = gp[:, m, :] if m < 8 else ps[:, m - 8, :]
                    nc.tensor.matmul(
                        out_ap,
                        wh_sb[:, (m * KH + k) * 128 : (m * KH + k + 1) * 128],
                        hb[:, k, :],
                        start=(m == 8 and k == 0),
                        stop=(k == KH - 1),
                        skip_group_check=True,
                    )

                # Gate math.  Critical chain:
                #   sigmoid_r(psum) -> rpn -> pre_n -> sig2 -> scan -> hb
                # where n = tanh(pre_n) = 2*sig2 - 1 with sig2 = sigmoid(2*pre_n),
                # so h' = (1-z)*n + z*h = a*sig2 + b with a = 2-2z and
                # b = z*h - 1 + z = z*(h+1) - 1 = z*hp1 - 1 (hp1 = h+1 was
                # computed last step, so b is 2 off-chain GpSimd ops after
                # sigmoid_z; a is one off-chain DVE op).  The blend a*sig2+b
                # is ONE DVE tensor_tensor_scan over interleaved pairs.
                rzt = ew.tile([128, 8, BL], F32, tag="rzt")
                nc.scalar.activation(
                    rzt[:, 0:KH, :], gp[:, 0:KH, :],
                    mybir.ActivationFunctionType.Sigmoid,
                )
                nc.scalar.activation(
                    rzt[:, KH : 2 * KH, :], gp[:, KH : 2 * KH, :],
                    mybir.ActivationFunctionType.Sigmoid,
                )
                d1 = ew.tile([128, KH, BL, 2], F32, tag="d1")
                zt = rzt[:, KH : 2 * KH, :]
                v = ew.tile([128, KH, BL], F32, tag="v")
                nc.gpsimd.tensor_mul(v[:], zt, h_prev)
                rpn = ew.tile([128, KH, BL], F32, tag="rpn")
                pre_n = ew.tile([128, KH, BL], F32, tag="pren")
                if use_bhn:
                    for k in range(KH):
                        nc.vector.scalar_tensor_tensor(
                            rpn[:, k, :],
                            ps[:, k, :],
                            bhn_sb[:, k : k + 1],
                            rzt[:, k, :],
                            mybir.AluOpType.add,
                            mybir.AluOpType.mult,
                        )
                    nc.vector.tensor_add(pre_n[:], rpn[:], gn_sb[:, t, :, :])
                else:
                    nc.vector.tensor_mul(rpn[:], ps[:], rzt[:, 0:KH, :])
                    nc.vector.tensor_add(pre_n[:], rpn[:], gn_sb[:, t, :, :])
                nc.vector.tensor_scalar(
                    d1[:, :, :, 0], zt, -2.0, 2.0,
                    mybir.AluOpType.mult, mybir.AluOpType.add,
                )
                # b = v - 1 + z on DVE (scalar_tensor_tensor is DVE-only)
                nc.vector.scalar_tensor_tensor(
                    d1[:, :, :, 1], v[:], -1.0, zt,
                    mybir.AluOpType.add, mybir.AluOpType.add,
                )
                d0 = d0_tiles[t % 2]
                nc.scalar.activation(
                    d0[:, :, :, 1], pre_n[:],
                    mybir.ActivationFunctionType.Sigmoid, scale=2.0,
                )
                hb4 = hbp.tile([128, KH, BL, 2], BF16, tag="hb")
                nc.vector.tensor_tensor_scan(
                    hb4.rearrange("p k j i -> p (k j i)"),
                    d0.rearrange("p k j i -> p (k j i)"),
                    d1.rearrange("p k j i -> p (k j i)"),
                    0.0,
                    mybir.AluOpType.mult,
                    mybir.AluOpType.add,
                )
                hb = hb4[:, :, :, 1]
                h_new = o_cur[:, :, u, :]
                # fp32 h for output/next-step b, off the critical chain:
                # h' = a*sig2 + b recomputed at f32 on GpSimd so these ops
                # never sit ahead of the scan in the DVE queue
                wq = ew.tile([128, KH, BL], F32, tag="wq")
                nc.gpsimd.tensor_mul(wq[:], d0[:, :, :, 1], d1[:, :, :, 0])
                nc.gpsimd.tensor_add(h_new, wq[:], d1[:, :, :, 1])
                h_prev = h_new

                # streamed gi precompute (chunks 1..CH-1) rides the PE/ACT
                # idle window; x-chunk DMAs issue a chunk ahead
                for c in stream_dma.get(t, ()):
                    load_x_chunk(c)
                for c, m, k in stream_mm.get(t, ()):
                    gi_mm(c, m, k, True)

                if u == RT - 1:
                    nc.sync.dma_start(
                        ys_v[:, :, t - RT + 1 : t + 1, :], o_cur[:]
                    )

    nc.compile()
    return nc


def _get(T, use_bi, use_bhn):
    key = (T, use_bi, use_bhn)
    if key not in _cache:
        _cache[key] = _build(T, use_bi, use_bhn)
    return _cache[key]


def _pack_w(W, kc):
    # W [kc*128, M3*128] -> [128, M3*kc*128], col ((m*kc)+k)*128+c = W[k*128+p, m*128+c]
    return np.ascontiguousarray(
        W.astype(NPBF16).reshape(kc, 128, M3, 128).transpose(1, 2, 0, 3).reshape(128, -1)
    )


def kernel(x, h0, Wi, Wh, bi, bhn, _trace=False, _trace_kwargs=None):
    T = x.shape[0]
    use_bi = bool(np.any(bi))
    use_bhn = bool(np.any(bhn))
    nc = _get(T, use_bi, use_bhn)
    TB = T * BL

    wi_p = _pack_w(np.asarray(Wi), KD)
    wh_p = _pack_w(np.asarray(Wh), KH)
    x = np.asarray(x)
    h0 = np.asarray(h0)

    in_maps = []
    for c in range(NCORES):
        xc = x[:, c * BL : (c + 1) * BL, :]  # [T, BL, D]
        xt = np.ascontiguousarray(
            xc.astype(NPBF16).reshape(T, BL, KD, 128).transpose(3, 2, 0, 1).reshape(128, KD * TB)
        )
        h0c = np.ascontiguousarray(
            h0[c * BL : (c + 1) * BL, :].astype(np.float32).reshape(BL, KH, 128).transpose(2, 1, 0).reshape(128, KH * BL)
        )
        im = {
            "xt": xt,
            "wi": wi_p,
            "wh": wh_p,
            "h0t": h0c,
            "eye": np.eye(128, dtype=np.float16),
        }
        if use_bi:
            im["bi_r"] = np.ascontiguousarray(bi.astype(NPBF16).reshape(1, M3 * 128))
        if use_bhn:
            im["bhn_t"] = np.ascontiguousarray(bhn.astype(np.float32).reshape(KH, 128).T)
        in_maps.append(im)

    kw = {}
    if _trace:
        kw = dict(trace=True, **(_trace_kwargs or {}))
    kernel._last_in_maps = in_maps
    res = bass_utils.run_bass_kernel_spmd(nc, in_maps, core_ids=list(range(NCORES)), **kw)

    ys = np.empty((T, B, H), dtype=np.float32)
    for c in range(NCORES):
        out = res.results[c]["yst"]  # [128, KH*TB]
        ys[:, c * BL : (c + 1) * BL, :] = (
            out.reshape(128, KH, T, BL).transpose(2, 3, 1, 0).reshape(T, BL, H)
        )
    kernel._last_result = res
    return ys
